# revision 12
# baseline (speedup 1.0000x reference)
"""Beltrami positional-encoding diffusion kernel for Trainium2 (8 NeuronCores).

Reference computation (per batch b):
    wx[y,x] = 1/(1 + 2*max(le[y,x], le[y,x-1]))      (circular)
    wy[y,x] = 1/(1 + 2*max(le[y,x], le[y-1,x]))
    5 diffusion steps on p (K=8 channels):
        dx = p - roll(p, 1, x);  gx = DT*wx*dx
        dy = p - roll(p, 1, y);  gy = DT*wy*dy
        p += gx(x+1) - gx + gy(y+1) - gy             (flux divergence, circular)

Sharding: 32 (b,k) planes over 8 cores -> 4 planes/core, one lambda plane/core.
Everything stays SBUF-resident in fp16 (fp32 PSUM accumulation).

SBUF plane layout: [128 partitions, NR+1 rows, W+2 cols] where image row
h = NR*partition + (row-1).  Row 0 is a circular top halo; col W duplicates
col 0 (horizontal wrap); col W+1 is slack so the fp16 row stride stays
4-byte aligned.  All bulk DVE ops run on flat contiguous views (2x mode);
the TensorEngine applies the flux divergence as +/-identity matmuls with
offset access patterns, accumulating p + div in fp32 PSUM.
"""

import sys

for _p in ("/opt/trn_rl_repo",):
    if _p not in sys.path:
        sys.path.insert(0, _p)

import numpy as np

ALPHA = 2.0
DT = 0.1
T_STEPS = 5

P = 128  # SBUF partitions
CHAIN_K = 9  # kernel invocations chained per dispatch in bench()


def build(H=1024, W=1024, nplanes=4, t_steps=T_STEPS):
    import concourse.mybir as mybir
    from concourse.bacc import Bacc
    from concourse.tile import TileContext

    f32 = mybir.dt.float32
    f16 = mybir.dt.float16

    NR = H // P           # image rows per partition
    WP = W + 2            # padded row width
    FL = NR * WP          # flat length of the image-row region per partition
    CH = 512 if W >= 512 else W  # matmul free-dim chunk (one PSUM bank)
    NCH = W // CH
    NHALF = 2 if NR % 2 == 0 else 1
    HNR = NR // NHALF     # rows per weight-setup pass

    nc = Bacc(None)
    p_in = nc.declare_dram_parameter("p_in", [nplanes, H, W], f32, isOutput=False)
    le_in = nc.declare_dram_parameter("le_in", [H, W], f32, isOutput=False)
    out = nc.declare_dram_parameter("out", [nplanes, H, W], f32, isOutput=True)

    ident_np = np.eye(P, dtype=np.float16)
    ident = nc.inline_tensor(ident_np, name="ident")
    nident = nc.inline_tensor(-ident_np, name="nident")

    # DRAM views in the partition layout: (P, NR, W)
    p_in_v = [p_in[i].rearrange("(p h) x -> p h x", h=NR) for i in range(nplanes)]
    le_v = le_in.rearrange("(p h) x -> p h x", h=NR)
    out_v = [out[i].rearrange("(p h) x -> p h x", h=NR) for i in range(nplanes)]

    def flat(t):
        return t.rearrange("p a b -> p (a b)")

    with TileContext(nc) as tc:
        with tc.tile_pool(name="pers", bufs=1) as pers:
            idt = pers.tile([P, P], f16, tag="idt")
            nidt = pers.tile([P, P], f16, tag="nidt")
            nc.sync.dma_start(out=idt[:, :], in_=ident[:, :])
            nc.sync.dma_start(out=nidt[:, :], in_=nident[:, :])

            wx = pers.tile([P, NR, WP], f16, tag="wx")
            wy = pers.tile([P, NR, WP], f16, tag="wy")
            pt = [
                pers.tile([P, NR + 1, WP], f16, tag=f"p{i}", name=f"pt{i}")
                for i in range(nplanes)
            ]

            # ------------- weights (one-time) + p loads (overlapped) -------------
            with tc.tile_pool(name="setup", bufs=1) as sp:
                le = sp.tile([P, NR + 1, WP], f32, tag="le")
                nc.sync.dma_start(out=le[:, 1 : NR + 1, 0:W], in_=le_v[:, :, :])
                # dup col + circular top halo derived on-chip / via 2 DMAs
                nc.scalar.copy(
                    out=le[:, 1 : NR + 1, W : W + 1], in_=le[:, 1 : NR + 1, 0:1]
                )
                nc.sync.dma_start(out=le[1:P, 0, 0:W], in_=le[0 : P - 1, NR, 0:W])
                nc.sync.dma_start(out=le[0:1, 0, 0:W], in_=le[P - 1 : P, NR, 0:W])

                lef = flat(le[:, :, :])
                for half in range(NHALF):
                    r0 = half * HNR
                    rows = slice((1 + r0) * WP, (1 + r0 + HNR) * WP)
                    for direction in range(2):  # 0: x (left), 1: y (up)
                        ta = sp.tile([P, HNR * WP], f32, tag="ta", name="ta")
                        tb = sp.tile([P, HNR * WP], f32, tag="tb", name="tb")
                        if direction == 0:
                            # skip flat elem 0 so in1 never reads the halo row
                            nc.vector.tensor_max(
                                out=ta[:, 1:],
                                in0=lef[:, rows.start + 1 : rows.stop],
                                in1=lef[:, rows.start : rows.stop - 1],
                            )
                            ta3 = ta[:, :].rearrange("p (a b) -> p a b", b=WP)
                            nc.vector.tensor_max(
                                out=ta3[:, :, 0:1],
                                in0=le[:, 1 + r0 : 1 + r0 + HNR, 0:1],
                                in1=le[:, 1 + r0 : 1 + r0 + HNR, W - 1 : W],
                            )
                        elif r0 == 0:
                            # row 0 reads the halo row; split it off
                            nc.vector.tensor_max(
                                out=ta[:, WP:],
                                in0=lef[:, rows.start + WP : rows.stop],
                                in1=lef[:, rows.start : rows.stop - WP],
                            )
                            nc.vector.tensor_max(
                                out=ta[:, 0:WP],
                                in0=lef[:, WP : 2 * WP],
                                in1=lef[:, 0:WP],
                            )
                        else:
                            nc.vector.tensor_max(
                                out=ta[:, :],
                                in0=lef[:, rows],
                                in1=lef[:, rows.start - WP : rows.stop - WP],
                            )
                        nc.vector.tensor_scalar(
                            out=tb[:, :],
                            in0=ta[:, :],
                            scalar1=ALPHA,
                            scalar2=1.0,
                            op0=mybir.AluOpType.mult,
                            op1=mybir.AluOpType.add,
                        )
                        tr = sp.tile([P, HNR * WP], f32, tag="ta", name="tr")
                        nc.vector.reciprocal_approx_fast(out=tr[:, :], in_=tb[:, :])
                        wf = flat((wx if direction == 0 else wy)[:, :, :])
                        nc.vector.tensor_scalar_mul(
                            out=wf[:, r0 * WP : (r0 + HNR) * WP],
                            in0=tr[:, :],
                            scalar1=DT,
                        )

                for i in range(nplanes):
                    stage = sp.tile([P, NR, W], f32, tag="le", name="stage")
                    nc.sync.dma_start(out=stage[:, :, :], in_=p_in_v[i][:, :, :])
                    nc.vector.tensor_copy(
                        out=pt[i][:, 1 : NR + 1, 0:W], in_=stage[:, :, :]
                    )
                    nc.scalar.copy(
                        out=pt[i][:, 1 : NR + 1, W : W + 1],
                        in_=pt[i][:, 1 : NR + 1, 0:1],
                    )
                    nc.sync.dma_start(
                        out=pt[i][1:P, 0, 0:W], in_=pt[i][0 : P - 1, NR, 0:W]
                    )
                    nc.sync.dma_start(
                        out=pt[i][0:1, 0, 0:W], in_=pt[i][P - 1 : P, NR, 0:W]
                    )

            tc.strict_bb_all_engine_barrier()

            wxf = flat(wx[:, :, :])
            wyf = flat(wy[:, :, :])

            # ------------- diffusion steps -------------
            with (
                tc.tile_pool(name="flux", bufs=2) as fpool,
                tc.tile_pool(name="psum", bufs=8, space="PSUM") as psum,
            ):
                for t_i in range(t_steps):
                    last = t_i == t_steps - 1
                    for i in range(nplanes):
                        ptf = flat(pt[i][:, :, :])

                        gxt = fpool.tile([P, NR, WP], f16, tag="gx", name="gxt")
                        gyt = fpool.tile([P, NR + 1, WP], f16, tag="gy", name="gyt")
                        gxf = flat(gxt)
                        gyf = flat(gyt)

                        # ps = p shifted right by one col (circular), via DMA,
                        # so the dx subtract keeps 4B alignment (2x mode)
                        pst = fpool.tile([P, NR, WP], f16, tag="ps", name="pst",
                                         bufs=1)
                        nc.sync.dma_start(
                            out=pst[:, :, 1 : W + 1],
                            in_=pt[i][:, 1 : NR + 1, 0:W],
                        )
                        nc.sync.dma_start(
                            out=pst[:, :, 0:1],
                            in_=pt[i][:, 1 : NR + 1, W - 1 : W],
                        )
                        psf = flat(pst)
                        # dx = p - ps  (both aligned -> 2x; wrap col correct)
                        nc.vector.tensor_sub(
                            out=gxf[:, 0:FL],
                            in0=ptf[:, WP : WP + FL],
                            in1=psf[:, 0:FL],
                        )
                        # gx = wx * dx  (in place; dup col is correct since
                        # p's dup col made dx's dup col right)
                        nc.vector.tensor_mul(
                            out=gxf[:, 0:FL], in0=wxf[:, 0:FL], in1=gxf[:, 0:FL]
                        )

                        # dy = p - p(y-1) into gyt rows (top halo = wrap)
                        if NR > 1:
                            nc.vector.tensor_sub(
                                out=gyf[:, WP:FL],
                                in0=ptf[:, 2 * WP : WP + FL],
                                in1=ptf[:, WP:FL],
                            )
                        nc.vector.tensor_sub(
                            out=gyf[:, 0:WP],
                            in0=ptf[:, WP : 2 * WP],
                            in1=ptf[:, 0:WP],
                        )
                        nc.vector.tensor_mul(
                            out=gyf[:, 0:FL], in0=wyf[:, 0:FL], in1=gyf[:, 0:FL]
                        )
                        # gy bottom halo row (image row below partition's last)
                        nc.sync.dma_start(
                            out=gyt[0 : P - 1, NR, 0:W], in_=gyt[1:P, 0, 0:W]
                        )
                        nc.sync.dma_start(
                            out=gyt[P - 1 : P, NR, 0:W], in_=gyt[0:1, 0, 0:W]
                        )

                        # p_new = p + gx(x+1) - gx + gy(y+1) - gy   (PSUM acc)
                        for r in range(NR):
                            for c in range(NCH):
                                x0 = c * CH
                                ps = psum.tile([P, CH], f32, tag="ps", name="ps")
                                mm = nc.tensor.matmul
                                mm(ps[:, :], idt[:, :], pt[i][:, 1 + r, x0 : x0 + CH],
                                   start=True, stop=False)
                                mm(ps[:, :], idt[:, :],
                                   gxt[:, r, x0 + 1 : x0 + 1 + CH],
                                   start=False, stop=False)
                                mm(ps[:, :], idt[:, :], gyt[:, r + 1, x0 : x0 + CH],
                                   start=False, stop=False)
                                mm(ps[:, :], nidt[:, :], gxt[:, r, x0 : x0 + CH],
                                   start=False, stop=False)
                                mm(ps[:, :], nidt[:, :], gyt[:, r, x0 : x0 + CH],
                                   start=False, stop=True)
                                nc.scalar.copy(
                                    out=pt[i][:, 1 + r, x0 : x0 + CH], in_=ps[:, :]
                                )

                        if last:
                            # fp16 -> fp32 casting DMA straight to HBM
                            nc.gpsimd.dma_start(
                                out=out_v[i][:, :, :], in_=pt[i][:, 1 : NR + 1, 0:W]
                            )
                        else:
                            # refresh dup col + top halo for the next step
                            nc.scalar.copy(
                                out=pt[i][:, 1 : NR + 1, W : W + 1],
                                in_=pt[i][:, 1 : NR + 1, 0:1],
                            )
                            nc.sync.dma_start(
                                out=pt[i][1:P, 0, 0:W], in_=pt[i][0 : P - 1, NR, 0:W]
                            )
                            nc.sync.dma_start(
                                out=pt[i][0:1, 0, 0:W], in_=pt[i][P - 1 : P, NR, 0:W]
                            )
    nc.compile()
    return nc


_CACHE = {}


def _get_nc(H, W, nplanes, t_steps=T_STEPS):
    key = (H, W, nplanes, t_steps)
    if key not in _CACHE:
        _CACHE[key] = build(H=H, W=W, nplanes=nplanes, t_steps=t_steps)
    return _CACHE[key]


def run(p_full, le_full, trace=False, t_steps=T_STEPS):
    """p_full: (B,K,H,W) f32, le_full: (B,1,H,W) f32 -> ((B,K,H,W) f32, exec_ns)."""
    from concourse.bass_utils import run_bass_kernel_spmd

    B, K, H, W = p_full.shape
    ncores = 8
    cpb = ncores // B          # cores per batch
    kpc = K // cpb             # channels per core
    nc = _get_nc(H, W, kpc, t_steps)

    in_maps = []
    for c in range(ncores):
        b = c // cpb
        k0 = (c % cpb) * kpc
        in_maps.append(
            {
                "p_in": np.ascontiguousarray(p_full[b, k0 : k0 + kpc]),
                "le_in": np.ascontiguousarray(le_full[b, 0]),
            }
        )
    res = run_bass_kernel_spmd(nc, in_maps, core_ids=list(range(ncores)), trace=trace)
    outp = np.empty((B, K, H, W), np.float32)
    for c in range(ncores):
        b = c // cpb
        k0 = (c % cpb) * kpc
        outp[b, k0 : k0 + kpc] = res.results[c]["out"]
    return outp, res.exec_time_ns


def bench(p_full, le_full, iters=10, t_steps=T_STEPS):
    """Time repeated on-device executions of the compiled kernel.

    Returns (outputs, times_s) where times_s are per-call wall times with
    inputs already resident on device (axon dispatch overhead included)."""
    import time

    import jax
    import jax.numpy as jnp
    from jax.sharding import Mesh, PartitionSpec
    from jax.experimental.shard_map import shard_map
    from concourse import bass2jax

    B, K, H, W = p_full.shape
    ncores = 8
    cpb = ncores // B
    kpc = K // cpb
    nc = _get_nc(H, W, kpc, t_steps)

    in_names = ["p_in", "le_in"]
    out_names = ["out"]
    out_avals = [jax.core.ShapedArray((kpc, H, W), jnp.float32)]
    n_params = 2

    partition_name = nc.partition_id_tensor.name if nc.partition_id_tensor else None
    all_in_names = in_names + out_names + ([partition_name] if partition_name else [])

    def _body(*args):
        operands = list(args)
        if partition_name is not None:
            operands.append(bass2jax.partition_id_tensor())
        outs = bass2jax._bass_exec_p.bind(
            *operands,
            out_avals=tuple(out_avals),
            in_names=tuple(all_in_names),
            out_names=tuple(out_names),
            lowering_input_output_aliases=(),
            sim_require_finite=True,
            sim_require_nnan=True,
            nc=nc,
        )
        return tuple(outs)

    devices = jax.devices()[:ncores]
    mesh = Mesh(np.asarray(devices), ("core",))
    in_specs = (PartitionSpec("core"),) * (n_params + 1)
    out_specs = (PartitionSpec("core"),)
    fn = jax.jit(
        shard_map(_body, mesh=mesh, in_specs=in_specs, out_specs=out_specs,
                  check_rep=False),
        keep_unused=True,
    )

    per_core_p = np.concatenate(
        [p_full[c // cpb, (c % cpb) * kpc : (c % cpb + 1) * kpc] for c in range(ncores)],
        axis=0,
    )
    per_core_le = np.concatenate(
        [le_full[c // cpb, 0] for c in range(ncores)], axis=0
    )
    zeros = np.zeros((ncores * kpc, H, W), np.float32)

    from jax.sharding import NamedSharding
    sh = NamedSharding(mesh, PartitionSpec("core"))
    d_p = jax.device_put(per_core_p, sh)
    d_le = jax.device_put(per_core_le, sh)
    d_z = jax.device_put(zeros, sh)

    out = fn(d_p, d_le, d_z)
    jax.block_until_ready(out)

    # second jit with many more diffusion steps: slope isolates device time
    nc_k = _get_nc(H, W, kpc, t_steps * CHAIN_K)

    def _body_k(*args):
        operands = list(args)
        if partition_name is not None:
            operands.append(bass2jax.partition_id_tensor())
        outs = bass2jax._bass_exec_p.bind(
            *operands,
            out_avals=tuple(out_avals),
            in_names=tuple(all_in_names),
            out_names=tuple(out_names),
            lowering_input_output_aliases=(),
            sim_require_finite=True,
            sim_require_nnan=True,
            nc=nc_k,
        )
        return tuple(outs)

    fnk = jax.jit(
        shard_map(_body_k, mesh=mesh, in_specs=in_specs,
                  out_specs=out_specs, check_rep=False),
        keep_unused=True,
    )
    jax.block_until_ready(fnk(d_p, d_le, d_z))

    t1s, tks = [], []
    for _ in range(iters):
        t0 = time.perf_counter()
        jax.block_until_ready(fn(d_p, d_le, d_z))
        t1s.append(time.perf_counter() - t0)
        t0 = time.perf_counter()
        jax.block_until_ready(fnk(d_p, d_le, d_z))
        tks.append(time.perf_counter() - t0)

    out_np = np.asarray(out[0]).reshape(ncores, kpc, H, W)
    outp = np.empty((B, K, H, W), np.float32)
    for c in range(ncores):
        outp[c // cpb, (c % cpb) * kpc : (c % cpb + 1) * kpc] = out_np[c]
    return outp, (t1s, tks)


def kernel(x, lambda_edge, gxx, gxy, gyy, p_init):
    p_full = np.asarray(p_init, np.float32)
    le_full = np.asarray(lambda_edge, np.float32)
    outp, _ = run(p_full, le_full, trace=False)
    return outp
